# revision 6
# baseline (speedup 1.0000x reference)
"""Trainium2 Bass kernel for nn_GumbelSigmoidMask (gnn_message_passing).

Strategy (edge-parallel across 8 cores, per sharding hint):
  - The edge MLP first layer decomposes: concat(x[src], x[dst]) @ W1e
    = (x @ W1e_top)[src] + (x @ W1e_bot)[dst].  Each core computes the
    per-node projection table pT[128 feats, N nodes] ONCE on the tensor
    engine (features on partitions: rows 0:64 = src-projection, rows
    64:128 = dst-projection), then gathers per-edge columns with the
    GPSIMD ap_gather instruction (SBUF->SBUF, free-dim gather, index
    groups 0-3 = src list, groups 4-7 = dst list -> one instruction
    fetches both endpoints of every edge with zero waste).
  - hidden = relu(g[0:64] + g[64:128])  (first-layer bias pre-folded
    into pT as b1e/2 per half), then the second layer is a PE matmul
    with a sliding-window stationary that places each 512-edge chunk's
    logits on its own PSUM partition row, so Gumbel-sigmoid runs on
    dense [128, 512] tiles.
  - Node branch is the same pipeline over x itself (replicated; host
    takes core 0's copy).
"""

import numpy as np
import ml_dtypes
from contextlib import ExitStack

import concourse.bass as bass
import concourse.bacc as bacc
import concourse.tile as tile
from concourse import mybir
from concourse.bass_utils import run_bass_kernel_spmd
from concourse.masks import make_identity

F32 = mybir.dt.float32
BF16 = mybir.dt.bfloat16
I16 = mybir.dt.int16
AF = mybir.ActivationFunctionType
ALU = mybir.AluOpType

P = 128
FEAT = 128
HID = 64
N_NODES = 20000
NODE_PAD = 20480          # 40 superchunks x 512
N_SC = 40                 # node superchunks (512 nodes each)
N_EDGES = 640000
CORES = 8
E_CORE = N_EDGES // CORES  # 80000
E_PAD = 81920              # 40 gather chunks x 2048
GCHUNK = 2048              # edges per ap_gather call
N_GC = E_PAD // GCHUNK     # 40
SCHUNK = 512               # edges per sigma-matmul (one PSUM bank row)
E_SCHUNKS = E_PAD // SCHUNK  # 160 -> bank A rows 0..127, bank B rows 0..31
EPS = 1e-8

_CACHE = {}


def _build():
    nc = bacc.Bacc(None, target_bir_lowering=False)

    x_in = nc.declare_dram_parameter("x", [N_NODES, FEAT], F32, isOutput=False)
    idx_in = nc.declare_dram_parameter("idx", [P, E_PAD // 16], I16, isOutput=False)
    ue_in = nc.declare_dram_parameter("ue", [E_SCHUNKS, SCHUNK], F32, isOutput=False)
    un_in = nc.declare_dram_parameter("un", [N_SC, SCHUNK], F32, isOutput=False)
    we_in = nc.declare_dram_parameter("w_edge", [FEAT, P], BF16, isOutput=False)
    wn_in = nc.declare_dram_parameter("w_node", [FEAT, HID], BF16, isOutput=False)
    wine_in = nc.declare_dram_parameter("win_e", [HID, 256], BF16, isOutput=False)
    winn_in = nc.declare_dram_parameter("win_n", [HID, 256], BF16, isOutput=False)
    beh_in = nc.declare_dram_parameter("b_edge_half", [P, 1], F32, isOutput=False)
    bn_in = nc.declare_dram_parameter("b_node", [HID, 1], F32, isOutput=False)
    b2e_in = nc.declare_dram_parameter("b2e", [P, 1], F32, isOutput=False)
    b2n_in = nc.declare_dram_parameter("b2n", [P, 1], F32, isOutput=False)

    eo_out = nc.declare_dram_parameter("edge_out", [E_SCHUNKS, SCHUNK], F32, isOutput=True)
    no_out = nc.declare_dram_parameter("node_out", [N_SC, SCHUNK], F32, isOutput=True)

    with tile.TileContext(nc) as tc, ExitStack() as ctx:
        singles = ctx.enter_context(tc.tile_pool(name="singles", bufs=1))
        xf_pool = ctx.enter_context(tc.tile_pool(name="xf", bufs=3))
        xb_pool = ctx.enter_context(tc.tile_pool(name="xb", bufs=2))
        xT_pool = ctx.enter_context(tc.tile_pool(name="xT", bufs=2))
        hn_pool = ctx.enter_context(tc.tile_pool(name="hn", bufs=2))
        g_pool = ctx.enter_context(tc.tile_pool(name="g", bufs=2))
        g2_pool = ctx.enter_context(tc.tile_pool(name="g2", bufs=2))
        hp_pool = ctx.enter_context(tc.tile_pool(name="hp", bufs=2))
        hs_pool = ctx.enter_context(tc.tile_pool(name="hs", bufs=2))
        misc = ctx.enter_context(tc.tile_pool(name="misc", bufs=2))
        ps_t = ctx.enter_context(tc.tile_pool(name="ps_t", bufs=2, space="PSUM"))
        ps_pf = ctx.enter_context(tc.tile_pool(name="ps_pf", bufs=2, space="PSUM"))
        ps_pn = ctx.enter_context(tc.tile_pool(name="ps_pn", bufs=1, space="PSUM"))
        ps_acc = ctx.enter_context(tc.tile_pool(name="ps_acc", bufs=1, space="PSUM"))

        # --- constants / tables ---
        we_sb = singles.tile([FEAT, P], BF16)
        nc.sync.dma_start(out=we_sb, in_=we_in[:, :])
        wn_sb = singles.tile([FEAT, HID], BF16)
        nc.sync.dma_start(out=wn_sb, in_=wn_in[:, :])
        wine_sb = singles.tile([HID, 256], BF16)
        nc.sync.dma_start(out=wine_sb, in_=wine_in[:, :])
        winn_sb = singles.tile([HID, 256], BF16)
        nc.sync.dma_start(out=winn_sb, in_=winn_in[:, :])
        beh_sb = singles.tile([P, 1], F32)
        nc.sync.dma_start(out=beh_sb, in_=beh_in[:, :])
        bn_sb = singles.tile([HID, 1], F32)
        nc.sync.dma_start(out=bn_sb, in_=bn_in[:, :])
        b2e_sb = singles.tile([P, 1], F32)
        nc.sync.dma_start(out=b2e_sb, in_=b2e_in[:, :])
        b2n_sb = singles.tile([P, 1], F32)
        nc.sync.dma_start(out=b2n_sb, in_=b2n_in[:, :])
        idx_sb = singles.tile([P, E_PAD // 16], I16)
        nc.sync.dma_start(out=idx_sb, in_=idx_in[:, :])
        ueA_sb = singles.tile([P, SCHUNK], F32)
        nc.sync.dma_start(out=ueA_sb, in_=ue_in[0:P, :])
        ueB_sb = singles.tile([32, SCHUNK], F32)
        nc.sync.dma_start(out=ueB_sb, in_=ue_in[P:E_SCHUNKS, :])
        un_sb = singles.tile([N_SC, SCHUNK], F32)
        nc.sync.dma_start(out=un_sb, in_=un_in[:, :])

        ident = singles.tile([P, P], BF16)
        make_identity(nc, ident[:, :])

        pT = singles.tile([P, NODE_PAD, 1], F32)

        # long-lived PSUM accumulators (one bank each)
        nacc = ps_acc.tile([P, SCHUNK], F32, tag="nacc")
        eaccA = ps_acc.tile([P, SCHUNK], F32, tag="eaccA")
        eaccB = ps_acc.tile([P, SCHUNK], F32, tag="eaccB")

        def gumbel_store(rows, u_sb, acc, b2_sb, out_dram):
            """mask = sigmoid(acc + log(u') - log(1-u') + b2), u'=clip(u)."""
            t0 = misc.tile([P, SCHUNK], F32, tag="t0")
            nc.vector.tensor_scalar(t0[:rows], u_sb[:rows], EPS, 1.0 - EPS,
                                    ALU.max, ALU.min)
            lnu = misc.tile([P, SCHUNK], F32, tag="lnu")
            nc.scalar.activation(lnu[:rows], t0[:rows], AF.Ln)
            t1 = misc.tile([P, SCHUNK], F32, tag="t1")
            nc.vector.tensor_scalar(t1[:rows], t0[:rows], -1.0, 1.0,
                                    ALU.mult, ALU.add)
            ln1mu = misc.tile([P, SCHUNK], F32, tag="ln1mu")
            nc.scalar.activation(ln1mu[:rows], t1[:rows], AF.Ln)
            noise = misc.tile([P, SCHUNK], F32, tag="noise")
            nc.vector.tensor_tensor(out=noise[:rows], in0=lnu[:rows],
                                    in1=ln1mu[:rows], op=ALU.subtract)
            logit = misc.tile([P, SCHUNK], F32, tag="logit")
            nc.vector.tensor_tensor(out=logit[:rows], in0=acc[:rows],
                                    in1=noise[:rows], op=ALU.add)
            mask = misc.tile([P, SCHUNK], F32, tag="mask")
            nc.scalar.activation(mask[:rows], logit[:rows], AF.Sigmoid,
                                 bias=b2_sb[:rows])
            nc.sync.dma_start(out=out_dram, in_=mask[:rows])

        # ---------------- phase 1: per-node projections + node branch ----
        for sc in range(N_SC):
            xf = xf_pool.tile([P, 4, FEAT], F32)
            r0 = sc * 512
            if r0 + 512 <= N_NODES:
                nc.sync.dma_start(
                    out=xf,
                    in_=x_in[r0:r0 + 512, :].rearrange("(j p) f -> p j f", p=P),
                )
            else:
                nc.vector.memset(xf, 0.0)
                rem = N_NODES - r0  # 32 valid rows in the last superchunk
                if rem > 0:
                    nc.sync.dma_start(out=xf[0:rem, 0, :], in_=x_in[r0:N_NODES, :])
            xb = xb_pool.tile([P, 4, FEAT], BF16)
            nc.vector.tensor_copy(out=xb, in_=xf)
            psT = ps_t.tile([P, 4, P], BF16)
            for j in range(4):
                nc.tensor.transpose(psT[:, j, :], xb[:, j, :], ident[:, :])
            xT = xT_pool.tile([P, 4, P], BF16)
            nc.vector.tensor_copy(out=xT, in_=psT)
            xT2 = xT.rearrange("k j n -> k (j n)")

            pf = ps_pf.tile([P, SCHUNK], F32)
            nc.tensor.matmul(pf, lhsT=we_sb[:, :], rhs=xT2, start=True, stop=True)
            nc.vector.tensor_scalar(pT[:, r0:r0 + 512, 0], pf, beh_sb[:, :], None,
                                    ALU.add)

            pn = ps_pn.tile([HID, SCHUNK], F32)
            nc.tensor.matmul(pn, lhsT=wn_sb[:, :], rhs=xT2, start=True, stop=True)
            hn = hn_pool.tile([HID, SCHUNK], BF16)
            nc.scalar.activation(hn, pn, AF.Relu, bias=bn_sb[:, :])
            nc.tensor.matmul(nacc, lhsT=winn_sb[:, 127 - sc:255 - sc], rhs=hn,
                             start=(sc == 0), stop=(sc == N_SC - 1),
                             skip_group_check=True)

        gumbel_store(N_SC, un_sb, nacc, b2n_sb, no_out[:, :])

        # ---------------- phase 2: edge gather + MLP ---------------------
        for gc in range(N_GC):
            g = g_pool.tile([P, GCHUNK, 1], F32)
            nc.gpsimd.ap_gather(
                out_ap=g[:, :, :],
                in_ap=pT[:, :, :],
                idxs_ap=idx_sb[:, gc * (GCHUNK // 16):(gc + 1) * (GCHUNK // 16)],
                channels=P,
                num_elems=NODE_PAD,
                d=1,
                num_idxs=GCHUNK,
            )
            g2 = g2_pool.tile([HID, GCHUNK], F32)
            nc.sync.dma_start(out=g2, in_=g[HID:P, :, 0])
            hp = hp_pool.tile([HID, GCHUNK], BF16)
            nc.vector.tensor_tensor(out=hp, in0=g[0:HID, :, 0],
                                    in1=g2, op=ALU.add)
            hs = hs_pool.tile([HID, GCHUNK], BF16)
            nc.scalar.activation(hs, hp, AF.Relu)
            for m in range(GCHUNK // SCHUNK):
                s = gc * (GCHUNK // SCHUNK) + m
                if s < P:
                    acc, srow = eaccA, s
                    start, stop = (s == 0), (s == P - 1)
                else:
                    acc, srow = eaccB, s - P
                    start, stop = (s == P), (s == E_SCHUNKS - 1)
                nc.tensor.matmul(acc, lhsT=wine_sb[:, 127 - srow:255 - srow],
                                 rhs=hs[:, m * SCHUNK:(m + 1) * SCHUNK],
                                 start=start, stop=stop, skip_group_check=True)

        gumbel_store(P, ueA_sb, eaccA, b2e_sb, eo_out[0:P, :])
        gumbel_store(32, ueB_sb, eaccB, b2e_sb, eo_out[P:E_SCHUNKS, :])

    nc.compile()
    return nc


def _get_nc():
    if "nc" not in _CACHE:
        _CACHE["nc"] = _build()
    return _CACHE["nc"]


def _wrap16(a):
    """index list [E_PAD] -> [16, E_PAD//16] with element i at (i%16, i//16)."""
    return np.ascontiguousarray(a.reshape(-1, 16).T)


def prepare_in_maps(x, edge_index, u_node, u_edge,
                    w1n, b1n, w2n, b2n, w1e, b1e, w2e, b2e):
    x = np.asarray(x, np.float32)
    edge_index = np.asarray(edge_index)
    u_node = np.asarray(u_node, np.float32)
    u_edge = np.asarray(u_edge, np.float32)
    bf = ml_dtypes.bfloat16

    w_edge = np.ascontiguousarray(
        np.concatenate([np.asarray(w1e)[:FEAT, :], np.asarray(w1e)[FEAT:, :]],
                       axis=1)).astype(bf)
    w_node = np.ascontiguousarray(np.asarray(w1n)).astype(bf)
    win_e = np.zeros((HID, 256), bf)
    win_e[:, 127] = np.asarray(w2e)[:, 0].astype(bf)
    win_n = np.zeros((HID, 256), bf)
    win_n[:, 127] = np.asarray(w2n)[:, 0].astype(bf)
    beh = np.concatenate([np.asarray(b1e), np.asarray(b1e)]).astype(np.float32)
    beh = (beh / 2.0).reshape(P, 1)
    bn = np.asarray(b1n, np.float32).reshape(HID, 1)
    b2e_t = np.full((P, 1), float(np.asarray(b2e)[0]), np.float32)
    b2n_t = np.full((P, 1), float(np.asarray(b2n)[0]), np.float32)

    un = np.full(NODE_PAD, 0.5, np.float32)
    un[:N_NODES] = u_node
    un = un.reshape(N_SC, SCHUNK)

    in_maps = []
    for c in range(CORES):
        sl = slice(c * E_CORE, (c + 1) * E_CORE)
        src = np.zeros(E_PAD, np.int16)
        dst = np.zeros(E_PAD, np.int16)
        src[:E_CORE] = edge_index[0, sl].astype(np.int16)
        dst[:E_CORE] = edge_index[1, sl].astype(np.int16)
        idx_t = np.empty((P, E_PAD // 16), np.int16)
        sw, dw = _wrap16(src), _wrap16(dst)
        for grp in range(8):
            idx_t[16 * grp:16 * grp + 16] = sw if grp < 4 else dw
        ue = np.full(E_PAD, 0.5, np.float32)
        ue[:E_CORE] = u_edge[sl]
        ue = ue.reshape(E_SCHUNKS, SCHUNK)
        in_maps.append({
            "x": x, "idx": idx_t, "ue": ue, "un": un,
            "w_edge": w_edge, "w_node": w_node,
            "win_e": win_e, "win_n": win_n,
            "b_edge_half": beh, "b_node": bn, "b2e": b2e_t, "b2n": b2n_t,
        })
    return in_maps


def assemble_outputs(results):
    node_mask = np.asarray(results[0]["node_out"], np.float32).reshape(-1)[:N_NODES]
    edge_mask = np.concatenate([
        np.asarray(results[c]["edge_out"], np.float32).reshape(-1)[:E_CORE]
        for c in range(CORES)
    ])
    return node_mask, edge_mask


def run(inputs, trace=False):
    nc = _get_nc()
    in_maps = prepare_in_maps(**inputs)
    res = run_bass_kernel_spmd(nc, in_maps, list(range(CORES)), trace=trace)
    return assemble_outputs(res.results), res


def kernel(**inputs):
    (node_mask, edge_mask), _ = run(inputs, trace=False)
    return node_mask, edge_mask
